# revision 39
# baseline (speedup 1.0000x reference)
"""Trainium2 Bass kernel for causal self-attention with doubled rotary.

Full-input contract: kernel(**inputs) takes the complete tensors
(x [4,2048,2048], wq/wk/wv/wo [2048,2048]) and returns [4,2048,2048] fp32.

Sharding: 8 cores = 4 batch elements x 2 head-halves (8 heads each).
Each core computes a partial output projection (its heads' columns of wo);
the host sums the two partials per batch element.

All matmul operands are fp16 (fp32 PSUM accumulation): same PE streaming
rate as fp32r but weight loads are half cost (and FWL-eligible), DVE ops
run in 2x packed mode, and DMA traffic halves. Verified numerically on the
reference inputs: rel err ~5e-4 vs the 2e-2 gate.

Per-core structure (engine streams execute in emission order, so
independent work is interleaved at emission time to keep the PE dense):
  - group g in 0..3 owns heads (2g, 2g+1): q/k/v projections in 512-wide
    t-panels (16 e-chunk matmuls per PSUM accumulation chain), doubled-angle
    rotary on DVE (the reference applies rotary twice; R(t)^2 == R(2t)).
  - attention pair g-1 is emitted interleaved with the projection of group
    g: QK^T computed transposed (ST[s,t]) so exp(ST) feeds the PV matmul
    directly with v as the stationary operand. exp runs on ACT with a
    constant bias -CEXP (cancels in softmax; keeps fp16 in range). Softmax
    denominators accumulate on DVE in fp16; one all-ones [128,128] matmul
    per (head, panel) does the partition reduce + broadcast; full-width
    reciprocal; normalization rides the PSUM->SBUF cast of the PV output.
    y tiles stay in SBUF (no DRAM spill) for the output projection.
  - the last pair is interleaved per-panel with the first half of the
    output projection; the rest of the output projection follows.
"""

import os
import sys

for _p in ("/opt/trn_rl_repo", "/root/.axon_site/_ro/trn_rl_repo"):
    if os.path.isdir(_p) and _p not in sys.path:
        sys.path.insert(0, _p)

import numpy as np

import concourse.bass as bass
import concourse.mybir as mybir
from concourse import bacc
from concourse import bass_isa
from concourse.bass import ds
from concourse.tile import TileContext
from concourse.bass_utils import run_bass_kernel_spmd

F32 = mybir.dt.float32
FP16 = mybir.dt.float16

P = 128          # partitions / head dim
T = 2048         # sequence length
E = 2048         # embedding dim
B = 4
HPC = 8          # heads per core
D = 128          # head dim
PAN = 512        # t-panel width (PSUM bank limit for fp32)
NPAN = T // PAN  # 4
EO = E // P      # 16 contraction chunks for projections
NGRP = 4         # head pairs per core
NCH = T // P     # 16 s-chunks (also v t-tiles)
SCALE = 1.0 / float(np.sqrt(D))
CEXP = 2.0       # exp bias: exp(scale*s - CEXP); cancels in softmax
NEGM = -30000.0  # additive causal mask value (fp16-safe)

ADD = mybir.AluOpType.add
MULT = mybir.AluOpType.mult
EXP = mybir.ActivationFunctionType.Exp


def _zip_emit(*lists):
    """Emit thunks from several lists round-robin, proportionally."""
    lists = [list(l) for l in lists if l]
    if not lists:
        return
    total = max(len(l) for l in lists)
    idx = [0.0] * len(lists)
    step = [len(l) / total for l in lists]
    for _ in range(total):
        for li, l in enumerate(lists):
            idx[li] += step[li]
            while idx[li] >= 1.0 and l:
                l.pop(0)()
                idx[li] -= 1.0
    for l in lists:
        for f in l:
            f()


class Ctx:
    pass


def build_program():
    nc = bacc.Bacc()
    cx = Ctx()
    cx.nc = nc

    cx.xT = nc.declare_dram_parameter("xT", [E, T], FP16, isOutput=False)
    cx.wqT = nc.declare_dram_parameter("wqT", [E, HPC * D], FP16, isOutput=False)
    cx.wkT = nc.declare_dram_parameter("wkT", [E, HPC * D], FP16, isOutput=False)
    cx.wvT = nc.declare_dram_parameter("wvT", [E, HPC * D], FP16, isOutput=False)
    cx.woT = nc.declare_dram_parameter("woT", [HPC * D, E], FP16, isOutput=False)
    cx.cos2 = nc.declare_dram_parameter("cos2", [P, T], FP16, isOutput=False)
    cx.sin2 = nc.declare_dram_parameter("sin2", [P, T], FP16, isOutput=False)
    cx.mask = nc.declare_dram_parameter("mask", [P, P], FP16, isOutput=False)
    cx.ident = nc.declare_dram_parameter("ident", [P, P], FP16, isOutput=False)
    cx.out = nc.declare_dram_parameter("out", [E, T], FP16, isOutput=True)

    with TileContext(nc) as tc:
        cx.tc = tc
        with tc.tile_pool(name="const", bufs=1) as cpool:
            cx.cpool = cpool
            cx.mk = cpool.tile([P, P], FP16, tag="mk")
            cx.idm = cpool.tile([P, P], FP16, tag="idm")
            om_f = cpool.tile([P, P], F32, tag="om_f")
            nc.vector.memset(om_f, 1.0)
            cx.onesmat = cpool.tile([P, P], FP16, tag="onesmat")
            nc.scalar.copy(cx.onesmat, om_f)
            cx.ebias = cpool.tile([P, 1], F32, tag="ebias")
            nc.vector.memset(cx.ebias, -CEXP)
            cx.c2 = cpool.tile([P, T], FP16, tag="c2")
            cx.s2 = cpool.tile([P, T], FP16, tag="s2")

            with (
                tc.tile_pool(name="ex", bufs=6) as expool,
                tc.tile_pool(name="ac", bufs=2) as accpool,
                tc.tile_pool(name="dn1", bufs=2) as dn1pool,
                tc.tile_pool(name="dn2", bufs=2) as dn2pool,
                tc.tile_pool(name="yb", bufs=1) as ypool,
                tc.tile_pool(name="psS", bufs=3, space="PSUM") as psS,
                tc.tile_pool(name="psY", bufs=3, space="PSUM") as psY,
                tc.tile_pool(name="qk", bufs=2) as qkpool,
                tc.tile_pool(name="vp", bufs=2) as vpool,
            ):
                cx.expool, cx.accpool, cx.dn1pool = expool, accpool, dn1pool
                cx.dn2pool = dn2pool
                cx.ypool = ypool
                cx.psS, cx.psY = psS, psY
                cx.qkpool, cx.vpool = qkpool, vpool
                cx.qkv = {}   # g -> (qT, kT, v_sb)
                cx.ytile = {}  # (h, jp) -> SBUF y tile [P, PAN] fp16

                with (
                    tc.tile_pool(name="xp", bufs=2) as xpool,
                    tc.tile_pool(name="wp", bufs=2) as wpool,
                    tc.tile_pool(name="rot", bufs=2) as rotpool,
                    tc.tile_pool(name="sw", bufs=2) as swpool,
                    tc.tile_pool(name="psP", bufs=2, space="PSUM") as psP,
                ):
                    cx.xpool, cx.wpool = xpool, wpool
                    cx.rotpool, cx.swpool, cx.psP = rotpool, swpool, psP

                    # warm up the PE (HAM un-throttle) while the first
                    # weight/x DMAs are in flight: dependency-free matmuls
                    # on the ones tile.
                    wps = cx.psP.tile([P, PAN], F32, tag="psP")
                    for _ in range(84):
                        nc.tensor.matmul(
                            wps[:, :P], lhsT=cx.onesmat, rhs=cx.onesmat,
                            start=True, stop=True,
                        )

                    g0 = _proj_thunks(cx, 0)
                    # late-emit const DMAs: first MMs only need wq + x chunk
                    def const_dmas():
                        nc.sync.dma_start(cx.mk, cx.mask[:, :])
                        nc.sync.dma_start(cx.idm, cx.ident[:, :])
                        nc.sync.dma_start(cx.c2, cx.cos2[:, :])
                        nc.sync.dma_start(cx.s2, cx.sin2[:, :])
                    g0.insert(9, const_dmas)
                    for f in g0:
                        f()
                    for g in range(1, NGRP):
                        proj = _proj_thunks(cx, g)
                        attn = _attn_thunks(cx, g - 1)
                        if g == NGRP - 1:
                            # pull pair-3 panel 0 into this phase so the
                            # endgame starts with a full pipeline
                            attn = attn + _attn_thunks(cx, g, only_jp=0)
                        _zip_emit(proj, attn)

                with (
                    tc.tile_pool(name="wo", bufs=1) as wopool,
                    tc.tile_pool(name="ob", bufs=6) as opool,
                    tc.tile_pool(name="psO", bufs=2, space="PSUM") as psO,
                ):
                    cx.wopool, cx.opool, cx.psO = wopool, opool, psO
                    cx.wo_half = {}
                    _load_wo_half(cx, 0)
                    _load_wo_half(cx, 1)
                    # pair-3 panel 0 was emitted during the group-3 phase;
                    # stagger the remaining panels against the outproj of
                    # the previous panel.
                    oproj0 = [_outproj_thunks(cx, 0, only_jp=jp)
                              for jp in range(NPAN)]
                    for jp in range(1, NPAN):
                        _zip_emit(_attn_thunks(cx, NGRP - 1, only_jp=jp),
                                  oproj0[jp - 1])
                    _zip_emit(oproj0[NPAN - 1], _outproj_thunks(cx, 1))

    nc.finalize()
    return nc


def _proj_thunks(cx, g):
    """Thunk list for group g's projections + rotary."""
    nc = cx.nc
    thunks = []

    wmap = cx.__dict__.setdefault("_wmap", {})

    def start_group():
        wq_sb = cx.wpool.tile([P, EO, 2 * D], FP16, tag="wq")
        wk_sb = cx.wpool.tile([P, EO, 2 * D], FP16, tag="wk")
        wv_sb = cx.wpool.tile([P, EO, 2 * D], FP16, tag="wv")
        qT = cx.qkpool.tile([P, 2, T], FP16, tag="qT")
        kT = cx.qkpool.tile([P, 2, T], FP16, tag="kT")
        v_sb = cx.vpool.tile([P, NCH, 2 * D], FP16, tag="v")
        cx.qkv[g] = (qT, kT, v_sb)
        wmap[g] = (wq_sb, wk_sb, wv_sb)

    def load_w(wi, nsplit=2):
        def f():
            w_sb = wmap[g][wi]
            src = (cx.wqT, cx.wkT, cx.wvT)[wi]
            r = src.rearrange("(eo p) d -> p eo d", p=P)
            step = EO // nsplit
            for h in range(nsplit):
                nc.sync.dma_start(
                    w_sb[:, ds(h * step, step), :],
                    r[:, ds(h * step, step), ds(g * 2 * D, 2 * D)],
                )
        return f

    thunks.append(start_group)

    state = {}

    def load_panel(pj):
        def f():
            xp = cx.xpool.tile([P, EO, PAN], FP16, tag="xp")
            r = cx.xT.rearrange("(eo p) t -> p eo t", p=P)
            for c in range(4):
                nc.sync.dma_start(
                    xp[:, ds(c * 4, 4), :],
                    r[:, ds(c * 4, 4), ds(pj * PAN, PAN)],
                )
            state[pj] = xp
        return f

    def qk_tile(pj, wi, hl):
        def f():
            xp = state[pj]
            w_sb = wmap[g][wi]
            dst = cx.qkv[g][wi]
            ps = cx.psP.tile([P, PAN], F32, tag="psP")
            for eo in range(EO):
                nc.tensor.matmul(
                    ps,
                    lhsT=w_sb[:, eo, ds(hl * D, D)],
                    rhs=xp[:, eo, :],
                    start=(eo == 0),
                    stop=(eo == EO - 1),
                )
            nc.scalar.copy(dst[:, hl, ds(pj * PAN, PAN)], ps)
        return f

    def v_tile(pj, tt):
        def f():
            xp = state[pj]
            wv_sb = wmap[g][2]
            v_sb = cx.qkv[g][2]
            ps = cx.psP.tile([P, PAN], F32, tag="psP")
            psv = ps[:, : 2 * D]
            for eo in range(EO):
                nc.tensor.matmul(
                    psv,
                    lhsT=xp[:, eo, ds(tt * P, P)],
                    rhs=wv_sb[:, eo, :],
                    start=(eo == 0),
                    stop=(eo == EO - 1),
                )
            nc.scalar.copy(v_sb[:, pj * (PAN // P) + tt, :], psv)
        return f

    def rot_panel(src_i, hl, pj):
        def f():
            src = cx.qkv[g][src_i]
            sl = ds(pj * PAN, PAN)
            qsw = cx.swpool.tile([P, PAN], FP16, tag="qsw")
            nc.sync.dma_start(qsw[0:64, :], src[64:128, hl, sl])
            nc.sync.dma_start(qsw[64:128, :], src[0:64, hl, sl])
            tmp = cx.rotpool.tile([P, PAN], FP16, tag="rtmp")
            nc.vector.tensor_tensor(tmp, qsw[:, :], cx.s2[:, sl], op=MULT)
            nc.vector.tensor_tensor(
                src[:, hl, sl], src[:, hl, sl], cx.c2[:, sl], op=MULT
            )
            nc.vector.tensor_tensor(src[:, hl, sl], src[:, hl, sl], tmp, op=ADD)
        return f

    for pj in range(NPAN):
        if pj == 0:
            thunks.append(load_w(0, nsplit=4 if g == 0 else 2))
            thunks.append(load_panel(0))
            for hl in range(2):
                thunks.append(qk_tile(0, 0, hl))
            thunks.append(load_w(1))
            for hl in range(2):
                thunks.append(qk_tile(0, 1, hl))
            thunks.append(load_w(2))
        else:
            thunks.append(load_panel(pj))
            for wi in range(2):
                for hl in range(2):
                    thunks.append(qk_tile(pj, wi, hl))
        for tt in range(PAN // P):
            thunks.append(v_tile(pj, tt))
        for src_i in range(2):
            for hl in range(2):
                thunks.append(rot_panel(src_i, hl, pj))
    return thunks


def _attn_thunks(cx, g, only_jp=None):
    """Thunk list for the attention of head pair g (heads 2g, 2g+1)."""
    nc = cx.nc
    thunks = []
    st8 = cx.__dict__.setdefault(f"_attn_state_{g}", {})

    def chunk(hl, jp, i):
        def f():
            qT, kT, v_sb = cx.qkv[g]
            nch = 4 * jp + 4
            if i == 0:
                ytp = cx.psY.tile([P, PAN], F32, tag="psY")
                acc = cx.accpool.tile([P, PAN], FP16, tag="acc")
                st8[(hl, jp)] = (ytp, acc)
            ytp, acc = st8[(hl, jp)]
            di = i - 4 * jp
            off = P * di if di > 0 else 0
            w = PAN - off
            st = cx.psS.tile([P, PAN], F32, tag="psS")
            stw = st[:, off:PAN]
            nc.tensor.matmul(
                stw,
                lhsT=kT[:, hl, ds(i * P, P)],
                rhs=qT[:, hl, ds(jp * PAN + off, w)],
                start=True,
                stop=True,
            )
            if di >= 0:
                nc.vector.tensor_tensor(
                    st[:, off:off + P], st[:, off:off + P], cx.mk, op=ADD
                )
            ex = cx.expool.tile([P, PAN], FP16, tag="ex")
            exw = ex[:, off:PAN]
            nc.scalar.activation(exw, stw, EXP, bias=cx.ebias[:, :], scale=SCALE)
            nc.tensor.matmul(
                ytp[:, off:PAN],
                lhsT=v_sb[:, i, ds(hl * D, D)],
                rhs=exw,
                start=(i == 0),
                stop=(i == nch - 1),
            )
            if i == 0:
                nc.vector.tensor_copy(acc, ex)
            else:
                nc.vector.tensor_tensor(
                    acc[:, off:PAN], acc[:, off:PAN], exw, op=ADD
                )
        return f

    def finalize(hl, jp):
        def f():
            h = 2 * g + hl
            ytp, acc = st8.pop((hl, jp))
            # partition reduce + broadcast of the denominator on GpSimd
            # (keeps it off the PE queue)
            dps = cx.dn2pool.tile([P, PAN], F32, tag="dps")
            nc.gpsimd.partition_all_reduce(
                dps, acc, channels=P, reduce_op=bass_isa.ReduceOp.add
            )
            rdb = cx.dn1pool.tile([P, PAN], F32, tag="rdb")
            nc.vector.reciprocal_approx_fast(out=rdb, in_=dps)
            yt = cx.ypool.tile([P, PAN], FP16, tag=f"y{h}_{jp}")
            nc.vector.tensor_tensor(yt, ytp, rdb, op=MULT)
            cx.ytile[(h, jp)] = yt
        return f

    jps = range(NPAN) if only_jp is None else [only_jp]
    for jp in jps:
        nch = 4 * jp + 4
        for hl in range(2):
            for i in range(nch):
                thunks.append(chunk(hl, jp, i))
            thunks.append(finalize(hl, jp))
    return thunks


def _load_wo_half(cx, half):
    # split along e so the first outproj e-tiles only wait on their quarter
    nc = cx.nc
    wo_sb = cx.wopool.tile([P, HPC, E // 2], FP16, tag=f"wo{half}")
    r = cx.woT.rearrange("(c p) e -> p c e", p=P)
    q = E // 8
    for c in range(4):
        nc.sync.dma_start(
            wo_sb[:, :, ds(c * q, q)],
            r[:, :, ds(half * (E // 2) + c * q, q)],
        )
    cx.wo_half[half] = wo_sb


def _outproj_thunks(cx, half, only_jp=None):
    """Thunk list for the output projection over e-tiles of one wo half."""
    nc = cx.nc
    thunks = []

    def etile(jp, et):
        def f():
            wo_sb = cx.wo_half[half]
            ps = cx.psO.tile([P, PAN], F32, tag="psO")
            for dc in range(HPC):
                nc.tensor.matmul(
                    ps,
                    lhsT=wo_sb[:, dc, ds((et - half * 8) * P, P)],
                    rhs=cx.ytile[(dc, jp)],
                    start=(dc == 0),
                    stop=(dc == HPC - 1),
                )
            ob = cx.opool.tile([P, PAN], FP16, tag="ob")
            nc.vector.tensor_copy(ob, ps)
            eng = nc.gpsimd if et % 2 == 0 else nc.sync
            eng.dma_start(
                cx.out[ds(et * P, P), ds(jp * PAN, PAN)], ob
            )
        return f

    jps = range(NPAN) if only_jp is None else [only_jp]
    for jp in jps:
        for et in range(half * 8, half * 8 + 8):
            thunks.append(etile(jp, et))
    return thunks


def make_tables():
    j = np.arange(0, D, 2, dtype=np.float64) / D
    inv_freq = 1.0 / (10000.0 ** j)
    t = np.arange(T, dtype=np.float64)
    fr = np.outer(t, inv_freq)                            # [T, 64]
    c2 = np.cos(2.0 * fr).T                               # [64, T]
    s2 = np.sin(2.0 * fr).T
    cos2 = np.concatenate([c2, c2], axis=0).astype(np.float16)
    sin2 = np.concatenate([s2, -s2], axis=0).astype(np.float16)
    return cos2, sin2


def make_mask():
    s = np.arange(P)[:, None]
    c = np.arange(P)[None, :]
    return np.where(s <= c, 0.0, NEGM).astype(np.float16)


def make_in_maps(x, wq, wk, wv, wo):
    cos2, sin2 = make_tables()
    mask = make_mask()
    in_maps = []
    for c in range(8):
        b, hh = c // 2, c % 2
        rows = slice(hh * HPC * D, (hh + 1) * HPC * D)
        in_maps.append({
            "xT": np.ascontiguousarray(x[b].T).astype(np.float16),
            "wqT": np.ascontiguousarray(wq[rows].T).astype(np.float16),
            "wkT": np.ascontiguousarray(wk[rows].T).astype(np.float16),
            "wvT": np.ascontiguousarray(wv[rows].T).astype(np.float16),
            "woT": np.ascontiguousarray(wo[:, rows].T).astype(np.float16),
            "cos2": cos2,
            "sin2": sin2,
            "mask": mask,
            "ident": np.eye(P, dtype=np.float16),
        })
    return in_maps


_PROGRAM_CACHE = {}


def get_program():
    if "nc" not in _PROGRAM_CACHE:
        _PROGRAM_CACHE["nc"] = build_program()
    return _PROGRAM_CACHE["nc"]


def kernel(x, wq, wk, wv, wo, _results_hook=None):
    x = np.asarray(x, dtype=np.float32)
    wq = np.asarray(wq, dtype=np.float32)
    wk = np.asarray(wk, dtype=np.float32)
    wv = np.asarray(wv, dtype=np.float32)
    wo = np.asarray(wo, dtype=np.float32)

    nc = get_program()
    in_maps = make_in_maps(x, wq, wk, wv, wo)
    res = run_bass_kernel_spmd(
        nc, in_maps, list(range(8)), tmpdir=os.environ.get("BASS_TMPDIR")
    )
    if _results_hook is not None:
        _results_hook(res)
    outs = [r["out"].astype(np.float32) for r in res.results]
    full = np.empty((B, T, E), dtype=np.float32)
    for b in range(B):
        full[b] = (outs[2 * b] + outs[2 * b + 1]).T
    return full


# revision 43
# speedup vs baseline: 1.0416x; 1.0416x over previous
"""Trainium2 Bass kernel for causal self-attention with doubled rotary.

Full-input contract: kernel(**inputs) takes the complete tensors
(x [4,2048,2048], wq/wk/wv/wo [2048,2048]) and returns [4,2048,2048] fp32.

Sharding: 8 cores = 4 batch elements x 2 head-halves (8 heads each).
Each core computes a partial output projection (its heads' columns of wo);
the host sums the two partials per batch element.

All matmul operands are fp16 (fp32 PSUM accumulation): same PE streaming
rate as fp32r but weight loads are half cost (and FWL-eligible), DVE ops
run in 2x packed mode, and DMA traffic halves. Verified numerically on the
reference inputs: rel err ~5e-4 vs the 2e-2 gate.

Per-core structure (engine streams execute in emission order, so
independent work is interleaved at emission time to keep the PE dense):
  - group g in 0..3 owns heads (2g, 2g+1): q/k/v projections in 512-wide
    t-panels (16 e-chunk matmuls per PSUM accumulation chain), doubled-angle
    rotary on DVE (the reference applies rotary twice; R(t)^2 == R(2t)).
  - attention pair g-1 is emitted interleaved with the projection of group
    g: QK^T computed transposed (ST[s,t]) so exp(ST) feeds the PV matmul
    directly with v as the stationary operand. exp runs on ACT with a
    constant bias -CEXP (cancels in softmax; keeps fp16 in range). Softmax
    denominators accumulate on DVE in fp16; one all-ones [128,128] matmul
    per (head, panel) does the partition reduce + broadcast; full-width
    reciprocal; normalization rides the PSUM->SBUF cast of the PV output.
    y tiles stay in SBUF (no DRAM spill) for the output projection.
  - the last pair is interleaved per-panel with the first half of the
    output projection; the rest of the output projection follows.
"""

import os
import sys

for _p in ("/opt/trn_rl_repo", "/root/.axon_site/_ro/trn_rl_repo"):
    if os.path.isdir(_p) and _p not in sys.path:
        sys.path.insert(0, _p)

import numpy as np

import concourse.bass as bass
import concourse.mybir as mybir
from concourse import bacc
from concourse.bass import ds
from concourse.tile import TileContext
from concourse.bass_utils import run_bass_kernel_spmd

F32 = mybir.dt.float32
FP16 = mybir.dt.float16

P = 128          # partitions / head dim
T = 2048         # sequence length
E = 2048         # embedding dim
B = 4
HPC = 8          # heads per core
D = 128          # head dim
PAN = 512        # t-panel width (PSUM bank limit for fp32)
NPAN = T // PAN  # 4
EO = E // P      # 16 contraction chunks for projections
NGRP = 4         # head pairs per core
NCH = T // P     # 16 s-chunks (also v t-tiles)
SCALE = 1.0 / float(np.sqrt(D))
CEXP = 2.0       # exp bias: exp(scale*s - CEXP); cancels in softmax
NEGM = -30000.0  # additive causal mask value (fp16-safe)

ADD = mybir.AluOpType.add
MULT = mybir.AluOpType.mult
EXP = mybir.ActivationFunctionType.Exp


def _zip_emit(*lists):
    """Emit thunks from several lists round-robin, proportionally."""
    lists = [list(l) for l in lists if l]
    if not lists:
        return
    total = max(len(l) for l in lists)
    idx = [0.0] * len(lists)
    step = [len(l) / total for l in lists]
    for _ in range(total):
        for li, l in enumerate(lists):
            idx[li] += step[li]
            while idx[li] >= 1.0 and l:
                l.pop(0)()
                idx[li] -= 1.0
    for l in lists:
        for f in l:
            f()


class Ctx:
    pass


def build_program():
    nc = bacc.Bacc()
    cx = Ctx()
    cx.nc = nc

    cx.xT = nc.declare_dram_parameter("xT", [E, T], FP16, isOutput=False)
    cx.wqT = nc.declare_dram_parameter("wqT", [E, HPC * D], FP16, isOutput=False)
    cx.wkT = nc.declare_dram_parameter("wkT", [E, HPC * D], FP16, isOutput=False)
    cx.wvT = nc.declare_dram_parameter("wvT", [E, HPC * D], FP16, isOutput=False)
    cx.woT = nc.declare_dram_parameter("woT", [HPC * D, E], FP16, isOutput=False)
    cx.cos2 = nc.declare_dram_parameter("cos2", [P, T], FP16, isOutput=False)
    cx.sin2 = nc.declare_dram_parameter("sin2", [P, T], FP16, isOutput=False)
    cx.mask = nc.declare_dram_parameter("mask", [P, P], FP16, isOutput=False)
    cx.ident = nc.declare_dram_parameter("ident", [P, P], FP16, isOutput=False)
    cx.out = nc.declare_dram_parameter("out", [E, T], FP16, isOutput=True)

    with TileContext(nc) as tc:
        cx.tc = tc
        with tc.tile_pool(name="const", bufs=1) as cpool:
            cx.cpool = cpool
            cx.mk = cpool.tile([P, P], FP16, tag="mk")
            cx.idm = cpool.tile([P, P], FP16, tag="idm")
            om_f = cpool.tile([P, P], F32, tag="om_f")
            nc.vector.memset(om_f, 1.0)
            cx.onesmat = cpool.tile([P, P], FP16, tag="onesmat")
            nc.scalar.copy(cx.onesmat, om_f)
            cx.ebias = cpool.tile([P, 1], F32, tag="ebias")
            nc.vector.memset(cx.ebias, -CEXP)
            cx.c2 = cpool.tile([P, T], FP16, tag="c2")
            cx.s2 = cpool.tile([P, T], FP16, tag="s2")

            with (
                tc.tile_pool(name="ex", bufs=6) as expool,
                tc.tile_pool(name="ac", bufs=2) as accpool,
                tc.tile_pool(name="dn1", bufs=2) as dn1pool,
                tc.tile_pool(name="yb", bufs=1) as ypool,
                tc.tile_pool(name="psS", bufs=2, space="PSUM") as psS,
                tc.tile_pool(name="psY", bufs=2, space="PSUM") as psY,
                tc.tile_pool(name="psD", bufs=2, space="PSUM") as psD,
                tc.tile_pool(name="qk", bufs=2) as qkpool,
                tc.tile_pool(name="vp", bufs=2) as vpool,
            ):
                cx.expool, cx.accpool, cx.dn1pool = expool, accpool, dn1pool
                cx.ypool = ypool
                cx.psS, cx.psY, cx.psD = psS, psY, psD
                cx.qkpool, cx.vpool = qkpool, vpool
                cx.qkv = {}   # g -> (qT, kT, v_sb)
                cx.ytile = {}  # (h, jp) -> SBUF y tile [P, PAN] fp16

                with (
                    tc.tile_pool(name="xp", bufs=2) as xpool,
                    tc.tile_pool(name="wp", bufs=2) as wpool,
                    tc.tile_pool(name="rot", bufs=2) as rotpool,
                    tc.tile_pool(name="sw", bufs=2) as swpool,
                    tc.tile_pool(name="psP", bufs=2, space="PSUM") as psP,
                ):
                    cx.xpool, cx.wpool = xpool, wpool
                    cx.rotpool, cx.swpool, cx.psP = rotpool, swpool, psP

                    # warm up the PE (HAM un-throttle) while the first
                    # weight/x DMAs are in flight: dependency-free matmuls
                    # on the ones tile.
                    wps = cx.psP.tile([P, PAN], F32, tag="psP")
                    for _ in range(84):
                        nc.tensor.matmul(
                            wps[:, :P], lhsT=cx.onesmat, rhs=cx.onesmat,
                            start=True, stop=True,
                        )

                    g0 = _proj_thunks(cx, 0)
                    # late-emit const DMAs: first MMs only need wq + x chunk
                    def const_dmas():
                        nc.sync.dma_start(cx.mk, cx.mask[:, :])
                        nc.sync.dma_start(cx.idm, cx.ident[:, :])
                        nc.sync.dma_start(cx.c2, cx.cos2[:, :])
                        nc.sync.dma_start(cx.s2, cx.sin2[:, :])
                    g0.insert(9, const_dmas)
                    for f in g0:
                        f()
                    for g in range(1, NGRP):
                        proj = _proj_thunks(cx, g)
                        attn = _attn_thunks(cx, g - 1)
                        if g == NGRP - 1:
                            # pull pair-3 panel 0 into this phase so the
                            # endgame starts with a full pipeline
                            attn = attn + _attn_thunks(cx, g, only_jp=0)
                        _zip_emit(proj, attn)

                with (
                    tc.tile_pool(name="wo", bufs=1) as wopool,
                    tc.tile_pool(name="ob", bufs=6) as opool,
                    tc.tile_pool(name="psO", bufs=2, space="PSUM") as psO,
                ):
                    cx.wopool, cx.opool, cx.psO = wopool, opool, psO
                    cx.wo_half = {}
                    _load_wo_half(cx, 0)
                    _load_wo_half(cx, 1)
                    # pair-3 panel 0 was emitted during the group-3 phase;
                    # stagger the remaining panels against the outproj of
                    # the previous panel.
                    oproj0 = [_outproj_thunks(cx, 0, only_jp=jp)
                              for jp in range(NPAN)]
                    for jp in range(1, NPAN):
                        _zip_emit(_attn_thunks(cx, NGRP - 1, only_jp=jp),
                                  oproj0[jp - 1])
                    _zip_emit(oproj0[NPAN - 1], _outproj_thunks(cx, 1))

    nc.finalize()
    return nc


def _proj_thunks(cx, g):
    """Thunk list for group g's projections + rotary."""
    nc = cx.nc
    thunks = []

    wmap = cx.__dict__.setdefault("_wmap", {})

    def start_group():
        wq_sb = cx.wpool.tile([P, EO, 2 * D], FP16, tag="wq")
        wk_sb = cx.wpool.tile([P, EO, 2 * D], FP16, tag="wk")
        wv_sb = cx.wpool.tile([P, EO, 2 * D], FP16, tag="wv")
        qT = cx.qkpool.tile([P, 2, T], FP16, tag="qT")
        kT = cx.qkpool.tile([P, 2, T], FP16, tag="kT")
        v_sb = cx.vpool.tile([P, NCH, 2 * D], FP16, tag="v")
        cx.qkv[g] = (qT, kT, v_sb)
        wmap[g] = (wq_sb, wk_sb, wv_sb)

    def load_w(wi, nsplit=2):
        def f():
            w_sb = wmap[g][wi]
            src = (cx.wqT, cx.wkT, cx.wvT)[wi]
            r = src.rearrange("(eo p) d -> p eo d", p=P)
            step = EO // nsplit
            for h in range(nsplit):
                nc.sync.dma_start(
                    w_sb[:, ds(h * step, step), :],
                    r[:, ds(h * step, step), ds(g * 2 * D, 2 * D)],
                )
        return f

    thunks.append(start_group)

    state = {}

    def load_panel(pj):
        def f():
            xp = cx.xpool.tile([P, EO, PAN], FP16, tag="xp")
            r = cx.xT.rearrange("(eo p) t -> p eo t", p=P)
            for c in range(4):
                nc.sync.dma_start(
                    xp[:, ds(c * 4, 4), :],
                    r[:, ds(c * 4, 4), ds(pj * PAN, PAN)],
                )
            state[pj] = xp
        return f

    def qk_tile(pj, wi, hl):
        def f():
            xp = state[pj]
            w_sb = wmap[g][wi]
            dst = cx.qkv[g][wi]
            ps = cx.psP.tile([P, PAN], F32, tag="psP")
            for eo in range(EO):
                nc.tensor.matmul(
                    ps,
                    lhsT=w_sb[:, eo, ds(hl * D, D)],
                    rhs=xp[:, eo, :],
                    start=(eo == 0),
                    stop=(eo == EO - 1),
                )
            nc.scalar.copy(dst[:, hl, ds(pj * PAN, PAN)], ps)
        return f

    def v_tile(pj, tt):
        def f():
            xp = state[pj]
            wv_sb = wmap[g][2]
            v_sb = cx.qkv[g][2]
            ps = cx.psP.tile([P, PAN], F32, tag="psP")
            psv = ps[:, : 2 * D]
            for eo in range(EO):
                nc.tensor.matmul(
                    psv,
                    lhsT=xp[:, eo, ds(tt * P, P)],
                    rhs=wv_sb[:, eo, :],
                    start=(eo == 0),
                    stop=(eo == EO - 1),
                )
            nc.scalar.copy(v_sb[:, pj * (PAN // P) + tt, :], psv)
        return f

    def rot_panel(src_i, hl, pj):
        def f():
            src = cx.qkv[g][src_i]
            sl = ds(pj * PAN, PAN)
            qsw = cx.swpool.tile([P, PAN], FP16, tag="qsw")
            nc.sync.dma_start(qsw[0:64, :], src[64:128, hl, sl])
            nc.sync.dma_start(qsw[64:128, :], src[0:64, hl, sl])
            tmp = cx.rotpool.tile([P, PAN], FP16, tag="rtmp")
            nc.vector.tensor_tensor(tmp, qsw[:, :], cx.s2[:, sl], op=MULT)
            nc.vector.tensor_tensor(
                src[:, hl, sl], src[:, hl, sl], cx.c2[:, sl], op=MULT
            )
            nc.vector.tensor_tensor(src[:, hl, sl], src[:, hl, sl], tmp, op=ADD)
        return f

    for pj in range(NPAN):
        if pj == 0:
            thunks.append(load_w(0, nsplit=4 if g == 0 else 2))
            thunks.append(load_panel(0))
            for hl in range(2):
                thunks.append(qk_tile(0, 0, hl))
            thunks.append(load_w(1))
            for hl in range(2):
                thunks.append(qk_tile(0, 1, hl))
            thunks.append(load_w(2))
        else:
            thunks.append(load_panel(pj))
            for wi in range(2):
                for hl in range(2):
                    thunks.append(qk_tile(pj, wi, hl))
        for tt in range(PAN // P):
            thunks.append(v_tile(pj, tt))
        for src_i in range(2):
            for hl in range(2):
                thunks.append(rot_panel(src_i, hl, pj))
    return thunks


def _attn_thunks(cx, g, only_jp=None):
    """Thunk list for the attention of head pair g (heads 2g, 2g+1)."""
    nc = cx.nc
    thunks = []
    st8 = cx.__dict__.setdefault(f"_attn_state_{g}", {})

    def chunk(hl, jp, i):
        def f():
            qT, kT, v_sb = cx.qkv[g]
            nch = 4 * jp + 4
            if i == 0:
                ytp = cx.psY.tile([P, PAN], F32, tag="psY")
                acc = cx.accpool.tile([P, PAN], FP16, tag="acc")
                st8[(hl, jp)] = (ytp, acc)
            ytp, acc = st8[(hl, jp)]
            di = i - 4 * jp
            off = P * di if di > 0 else 0
            w = PAN - off
            st = cx.psS.tile([P, PAN], F32, tag="psS")
            stw = st[:, off:PAN]
            nc.tensor.matmul(
                stw,
                lhsT=kT[:, hl, ds(i * P, P)],
                rhs=qT[:, hl, ds(jp * PAN + off, w)],
                start=True,
                stop=True,
            )
            if di >= 0:
                nc.vector.tensor_tensor(
                    st[:, off:off + P], st[:, off:off + P], cx.mk, op=ADD
                )
            ex = cx.expool.tile([P, PAN], FP16, tag="ex")
            exw = ex[:, off:PAN]
            nc.scalar.activation(exw, stw, EXP, bias=cx.ebias[:, :], scale=SCALE)
            nc.tensor.matmul(
                ytp[:, off:PAN],
                lhsT=v_sb[:, i, ds(hl * D, D)],
                rhs=exw,
                start=(i == 0),
                stop=(i == nch - 1),
            )
            if i == 0:
                nc.vector.tensor_copy(acc, ex)
            else:
                nc.vector.tensor_tensor(
                    acc[:, off:PAN], acc[:, off:PAN], exw, op=ADD
                )
        return f

    def finalize(hl, jp):
        def f():
            h = 2 * g + hl
            ytp, acc = st8.pop((hl, jp))
            dps = cx.psD.tile([P, PAN], F32, tag="psD")
            nc.tensor.matmul(dps, lhsT=cx.onesmat, rhs=acc, start=True, stop=True)
            rdb = cx.dn1pool.tile([P, PAN], F32, tag="rdb")
            nc.vector.reciprocal_approx_fast(out=rdb, in_=dps)
            yt = cx.ypool.tile([P, PAN], FP16, tag=f"y{h}_{jp}")
            nc.vector.tensor_tensor(yt, ytp, rdb, op=MULT)
            cx.ytile[(h, jp)] = yt
        return f

    jps = range(NPAN) if only_jp is None else [only_jp]
    for jp in jps:
        nch = 4 * jp + 4
        for hl in range(2):
            for i in range(nch):
                thunks.append(chunk(hl, jp, i))
            thunks.append(finalize(hl, jp))
    return thunks


def _load_wo_half(cx, half):
    # split along e so the first outproj e-tiles only wait on their quarter
    nc = cx.nc
    wo_sb = cx.wopool.tile([P, HPC, E // 2], FP16, tag=f"wo{half}")
    r = cx.woT.rearrange("(c p) e -> p c e", p=P)
    q = E // 8
    for c in range(4):
        nc.sync.dma_start(
            wo_sb[:, :, ds(c * q, q)],
            r[:, :, ds(half * (E // 2) + c * q, q)],
        )
    cx.wo_half[half] = wo_sb


def _outproj_thunks(cx, half, only_jp=None):
    """Thunk list for the output projection over e-tiles of one wo half."""
    nc = cx.nc
    thunks = []

    def etile(jp, et):
        def f():
            wo_sb = cx.wo_half[half]
            ps = cx.psO.tile([P, PAN], F32, tag="psO")
            for dc in range(HPC):
                nc.tensor.matmul(
                    ps,
                    lhsT=wo_sb[:, dc, ds((et - half * 8) * P, P)],
                    rhs=cx.ytile[(dc, jp)],
                    start=(dc == 0),
                    stop=(dc == HPC - 1),
                )
            ob = cx.opool.tile([P, PAN], FP16, tag="ob")
            nc.vector.tensor_copy(ob, ps)
            eng = nc.gpsimd if et % 2 == 0 else nc.sync
            eng.dma_start(
                cx.out[ds(et * P, P), ds(jp * PAN, PAN)], ob
            )
        return f

    jps = range(NPAN) if only_jp is None else [only_jp]
    for jp in jps:
        for et in range(half * 8, half * 8 + 8):
            thunks.append(etile(jp, et))
    return thunks


def make_tables():
    j = np.arange(0, D, 2, dtype=np.float64) / D
    inv_freq = 1.0 / (10000.0 ** j)
    t = np.arange(T, dtype=np.float64)
    fr = np.outer(t, inv_freq)                            # [T, 64]
    c2 = np.cos(2.0 * fr).T                               # [64, T]
    s2 = np.sin(2.0 * fr).T
    cos2 = np.concatenate([c2, c2], axis=0).astype(np.float16)
    sin2 = np.concatenate([s2, -s2], axis=0).astype(np.float16)
    return cos2, sin2


def make_mask():
    s = np.arange(P)[:, None]
    c = np.arange(P)[None, :]
    return np.where(s <= c, 0.0, NEGM).astype(np.float16)


def make_in_maps(x, wq, wk, wv, wo):
    cos2, sin2 = make_tables()
    mask = make_mask()
    in_maps = []
    for c in range(8):
        b, hh = c // 2, c % 2
        rows = slice(hh * HPC * D, (hh + 1) * HPC * D)
        in_maps.append({
            "xT": np.ascontiguousarray(x[b].T).astype(np.float16),
            "wqT": np.ascontiguousarray(wq[rows].T).astype(np.float16),
            "wkT": np.ascontiguousarray(wk[rows].T).astype(np.float16),
            "wvT": np.ascontiguousarray(wv[rows].T).astype(np.float16),
            "woT": np.ascontiguousarray(wo[:, rows].T).astype(np.float16),
            "cos2": cos2,
            "sin2": sin2,
            "mask": mask,
            "ident": np.eye(P, dtype=np.float16),
        })
    return in_maps


_PROGRAM_CACHE = {}


def get_program():
    if "nc" not in _PROGRAM_CACHE:
        _PROGRAM_CACHE["nc"] = build_program()
    return _PROGRAM_CACHE["nc"]


def kernel(x, wq, wk, wv, wo, _results_hook=None):
    x = np.asarray(x, dtype=np.float32)
    wq = np.asarray(wq, dtype=np.float32)
    wk = np.asarray(wk, dtype=np.float32)
    wv = np.asarray(wv, dtype=np.float32)
    wo = np.asarray(wo, dtype=np.float32)

    nc = get_program()
    in_maps = make_in_maps(x, wq, wk, wv, wo)
    res = run_bass_kernel_spmd(
        nc, in_maps, list(range(8)), tmpdir=os.environ.get("BASS_TMPDIR")
    )
    if _results_hook is not None:
        _results_hook(res)
    outs = [r["out"].astype(np.float32) for r in res.results]
    full = np.empty((B, T, E), dtype=np.float32)
    for b in range(B):
        full[b] = (outs[2 * b] + outs[2 * b + 1]).T
    return full
